# revision 2
# baseline (speedup 1.0000x reference)
"""Cached-attention kernel v2 for Trainium2 (8 NeuronCores, Bass/Tile).

Problem: B=4, L=2048 new tokens, S=2048 cached tokens, D=2048.
  Q = x @ Wq.T ; K = x @ Wk.T ; V = x @ Wv.T
  K_cal = concat(K, cache_k) ; V_cal = concat(V, cache_v)
  out = softmax(Q @ K_cal.T / sqrt(D)) @ V_cal

Sharding: 8 cores = (batch b in 0..3) x (key-half h in 0..1). Each core
runs ALL 2048 queries of its batch against its 2048 local keys (1024
cached + 1024 new). Softmax is flash-style un-normalized: the core
returns numerator^T [D, L] (bf16) and denominator [1, L] (f32); the
host sums the two halves and divides.

v2 vs v1 (which round-tripped Q/K/V through DRAM in fp32, ~200MB DMA):
  - All matmul operands are bf16 (fp32/f32r may not mix with 16-bit
    operands on the PE, and bf16 halves both DMA bytes and SBUF bytes).
    PSUM accumulation stays fp32.
  - K^T and V (2048 local keys) live in SBUF for the whole kernel;
    K/V projections write straight into the resident tiles.
  - Q is projected per query-half into a resident Q^T tile and consumed
    by the fused attention loop; no DRAM round-trip.
  - x arrives with the core's kv-half as the FIRST 1024 columns (host
    permutes; output is un-permuted on host), so one shared program
    works for both h=0 and h=1 cores.
  - W matrices arrive host-packed as [128, et, dt, 128] so each weight
    tile DMA moves 4KB contiguous runs per partition.
  - Total DMA ~60MB/core vs ~200MB in v1.
"""

import numpy as np
import ml_dtypes

import concourse.bass as bass
import concourse.tile as tile
from concourse import bacc, mybir
from concourse import bass2jax

F32 = mybir.dt.float32
BF16 = mybir.dt.bfloat16
BF16_NP = ml_dtypes.bfloat16

D = 2048          # model dim
L = 2048          # new tokens (queries)
HALF = 1024       # per-core share of new/cached keys; also query-half
NT = D // 128     # 16 tiles of 128 along D/E/S
SCALE = 1.0 / float(np.sqrt(D))
N_CORES = 8

_NC_CACHE = {}


def build_program(reps=1):
    key = ("nc", reps)
    if key in _NC_CACHE:
        return _NC_CACHE[key]
    nc = bacc.Bacc(None, target_bir_lowering=False, debug=False)
    xT = nc.dram_tensor("xT", [D, L], BF16, kind="ExternalInput")
    wq = nc.dram_tensor("wq", [128, NT, NT, 128], BF16, kind="ExternalInput")
    wk = nc.dram_tensor("wk", [128, NT, NT, 128], BF16, kind="ExternalInput")
    wvT = nc.dram_tensor("wvT", [D, D], BF16, kind="ExternalInput")
    kcT = nc.dram_tensor("kcT", [D, HALF], BF16, kind="ExternalInput")
    vc = nc.dram_tensor("vc", [HALF, D], BF16, kind="ExternalInput")
    outT = nc.dram_tensor("outT", [D, L], BF16, kind="ExternalOutput")
    den = nc.dram_tensor("den", [1, L], F32, kind="ExternalOutput")

    from contextlib import ExitStack
    with tile.TileContext(nc) as tc:
        with ExitStack() as _rep_stack:
            if reps > 1:
                _rep_stack.enter_context(
                    tc.For_i(0, reps, 1, hint_engines=tuple(mybir.EngineType))
                )
            _emit_body(nc, tc, xT, wq, wk, wvT, kcT, vc, outT, den)
    nc.compile()
    _NC_CACHE[key] = nc
    return nc


def _emit_body(nc, tc, xT, wq, wk, wvT, kcT, vc, outT, den):
    xT_r = xT.rearrange("(t p) l -> p t l", p=128)
    wvT_r = wvT.rearrange("(t p) d -> p t d", p=128)
    kcT_r = kcT.rearrange("(t p) s -> p t s", p=128)
    vc_r = vc.rearrange("(t p) d -> p t d", p=128)

    with (
        tc.tile_pool(name="res", bufs=1) as rpool,
        tc.tile_pool(name="cst", bufs=1) as cpool,
    ):
        # SBUF residents (bf16): K^T [e, s_local], V [s_local, d], Q^T half.
        # K^T s_local: cols 0..1023 cached, 1024..2047 new.
        # V   s_local: tiles 0..7 cached, 8..15 new.
        kT = rpool.tile([128, NT, 2 * HALF], BF16, tag="kT")
        v = rpool.tile([128, NT, D], BF16, tag="v")

        ones_f = cpool.tile([128, 1], F32, tag="ones_f")
        nc.gpsimd.memset(ones_f[:], 1.0)
        ones = cpool.tile([128, 1], BF16, tag="ones")
        nc.vector.tensor_copy(ones[:], ones_f[:])

        # ---- K/V projections for the new half (x columns 0..1023) ----
        # (cached K/V resident loads are emitted mid-phase: they are not
        # needed until attention, and must not clog the DMA queue ahead
        # of the first projection operands)
        with (
            tc.tile_pool(name="xkv", bufs=2) as xkpool,
            tc.tile_pool(name="wkp", bufs=2) as wkpool,
            tc.tile_pool(name="wvp", bufs=2) as wvpool,
            tc.tile_pool(name="psKV", bufs=4, space="PSUM") as pskv,
        ):
            xkv = []
            for sc in range(2):
                t = xkpool.tile([128, NT, 512], BF16, tag="xkv")
                nc.sync.dma_start(t[:], xT_r[:, :, sc * 512:(sc + 1) * 512])
                xkv.append(t)
            # K_new^T[e, s]: lhsT = wk tile (stationary), rhs = x chunk
            for et in range(NT):
                w_sb = wkpool.tile([128, NT, 128], BF16, tag="wk")
                nc.sync.dma_start(w_sb[:], wk[:, et, :, :])
                for sc in range(2):
                    ps = pskv.tile([128, 512], F32, tag="ps")
                    for dt in range(NT):
                        nc.tensor.matmul(
                            ps[:], w_sb[:, dt, :], xkv[sc][:, dt, :],
                            start=(dt == 0), stop=(dt == NT - 1),
                        )
                    nc.vector.tensor_copy(
                        kT[:, et, HALF + sc * 512:HALF + (sc + 1) * 512], ps[:])
            # ---- cached K^T and V -> residents (needed from attention on) ----
            nc.sync.dma_start(kT[:, :, 0:HALF], kcT_r[:, :, :])
            nc.sync.dma_start(v[:, 0:8, :], vc_r[:, :, :])
            # V_new[s, d]: lhsT = x chunk cols (stationary), rhs = wvT chunk
            for dc in range(4):
                wv_sb = wvpool.tile([128, NT, 512], BF16, tag="wv")
                nc.sync.dma_start(wv_sb[:], wvT_r[:, :, dc * 512:(dc + 1) * 512])
                for st8 in range(8):
                    sc, so = divmod(st8, 4)
                    ps = pskv.tile([128, 512], F32, tag="ps")
                    for dt in range(NT):
                        nc.tensor.matmul(
                            ps[:], xkv[sc][:, dt, so * 128:(so + 1) * 128],
                            wv_sb[:, dt, :],
                            start=(dt == 0), stop=(dt == NT - 1),
                        )
                    nc.vector.tensor_copy(
                        v[:, 8 + st8, dc * 512:(dc + 1) * 512], ps[:])

        # ---- per query-half: Q projection then fused attention ----
        for lh in range(2):
            lo = lh * HALF
            with tc.tile_pool(name="qt", bufs=1) as qpool:
                qt = qpool.tile([128, NT, HALF], BF16, tag="qt")
                with (
                    tc.tile_pool(name="xq", bufs=2) as xqpool,
                    tc.tile_pool(name="wqp", bufs=2) as wqpool,
                    tc.tile_pool(name="psQ", bufs=4, space="PSUM") as psq,
                ):
                    xq = []
                    for lc in range(2):
                        t = xqpool.tile([128, NT, 512], BF16, tag="xq")
                        nc.sync.dma_start(
                            t[:], xT_r[:, :, lo + lc * 512:lo + (lc + 1) * 512])
                        xq.append(t)
                    for et in range(NT):
                        w_sb = wqpool.tile([128, NT, 128], BF16, tag="wq")
                        nc.sync.dma_start(w_sb[:], wq[:, et, :, :])
                        for lc in range(2):
                            ps = psq.tile([128, 512], F32, tag="ps")
                            for dt in range(NT):
                                nc.tensor.matmul(
                                    ps[:], w_sb[:, dt, :], xq[lc][:, dt, :],
                                    start=(dt == 0), stop=(dt == NT - 1),
                                )
                            nc.vector.tensor_copy(
                                qt[:, et, lc * 512:(lc + 1) * 512], ps[:])

                # attention on 512-query blocks of this half
                for lbc in range(2):
                    lbo = lbc * 512
                    gl = lo + lbo
                    with (
                        tc.tile_pool(name="pT", bufs=1) as ppool,
                        tc.tile_pool(name="oA", bufs=4) as opool,
                        tc.tile_pool(name="psS", bufs=3, space="PSUM") as psS,
                        tc.tile_pool(name="psO", bufs=3, space="PSUM") as psO,
                        tc.tile_pool(name="psD", bufs=1, space="PSUM") as psD,
                    ):
                        pT = ppool.tile([128, NT, 512], BF16, tag="pT")
                        # scores^T [s, l] -> p = exp(scale * s)
                        for st in range(NT):
                            ps = psS.tile([128, 512], F32, tag="psS")
                            for et in range(NT):
                                nc.tensor.matmul(
                                    ps[:],
                                    kT[:, et, st * 128:(st + 1) * 128],
                                    qt[:, et, lbo:lbo + 512],
                                    start=(et == 0), stop=(et == NT - 1),
                                )
                            nc.scalar.activation(
                                pT[:, st, :], ps[:],
                                mybir.ActivationFunctionType.Exp, scale=SCALE,
                            )
                        # numerator^T [d, l]
                        for dt in range(NT):
                            ps_o = psO.tile([128, 512], F32, tag="psO")
                            for st in range(NT):
                                nc.tensor.matmul(
                                    ps_o[:],
                                    v[:, st, dt * 128:(dt + 1) * 128],
                                    pT[:, st, :],
                                    start=(st == 0), stop=(st == NT - 1),
                                )
                            o_sb = opool.tile([128, 512], BF16, tag="o")
                            nc.vector.tensor_copy(o_sb[:], ps_o[:])
                            nc.sync.dma_start(
                                outT[dt * 128:(dt + 1) * 128, gl:gl + 512],
                                o_sb[:],
                            )
                        # denominator [1, l]
                        ps_d = psD.tile([1, 512], F32, tag="psD")
                        for st in range(NT):
                            nc.tensor.matmul(
                                ps_d[:], ones[:], pT[:, st, :],
                                start=(st == 0), stop=(st == NT - 1),
                            )
                        d_sb = opool.tile([1, 512], F32, tag="d")
                        nc.vector.tensor_copy(d_sb[:], ps_d[:])
                        nc.sync.dma_start(den[0:1, gl:gl + 512], d_sb[:])


def _pack_w(wT):
    """[d, e] -> [p, et, dt, 128] with d = dt*128+p, e = et*128+e_lo."""
    return np.ascontiguousarray(
        wT.reshape(NT, 128, NT, 128).transpose(1, 2, 0, 3))


def make_in_maps(x, cache_k, cache_v, Wq, Wk, Wv):
    """Per-core inputs. Core c = (b, h), b = c // 2, h = c % 2. The x
    columns are permuted so the kv-half comes first."""
    f32 = np.float32
    wq_p = _pack_w(np.asarray(Wq, f32).T.astype(BF16_NP))
    wk_p = _pack_w(np.asarray(Wk, f32).T.astype(BF16_NP))
    wvT = np.ascontiguousarray(np.asarray(Wv, f32).T).astype(BF16_NP)
    in_maps = []
    for c in range(N_CORES):
        b, h = divmod(c, 2)
        xb = np.asarray(x[b], f32)
        sl = slice(h * HALF, (h + 1) * HALF)
        ot = slice((1 - h) * HALF, (2 - h) * HALF)
        x_perm = np.concatenate([xb[sl], xb[ot]], axis=0)  # [L, D], kv half first
        in_maps.append({
            "xT": np.ascontiguousarray(x_perm.T).astype(BF16_NP),
            "wq": wq_p,
            "wk": wk_p,
            "wvT": wvT,
            "kcT": np.ascontiguousarray(
                np.asarray(cache_k[b, sl], f32).T).astype(BF16_NP),
            "vc": np.asarray(cache_v[b, sl], f32).astype(BF16_NP),
        })
    return in_maps


def combine(results):
    """out[b] = ((num_h0 + num_h1) / (den_h0 + den_h1)).T, undoing the
    per-core query permutation (core (b,h) processed tokens
    [h*HALF:(h+1)*HALF] first)."""
    B = N_CORES // 2
    out = np.empty((B, L, D), np.float32)
    num = np.empty((D, L), np.float64)
    dent = np.empty(L, np.float64)
    for b in range(B):
        r0, r1 = results[2 * b], results[2 * b + 1]
        # core (b,0): queries in natural order; core (b,1): halves swapped
        n0 = np.asarray(r0["outT"], np.float64)
        n1 = np.asarray(r1["outT"], np.float64)
        d0 = np.asarray(r0["den"][0], np.float64)
        d1 = np.asarray(r1["den"][0], np.float64)
        num[:, 0:HALF] = n0[:, 0:HALF] + n1[:, HALF:L]
        num[:, HALF:L] = n0[:, HALF:L] + n1[:, 0:HALF]
        dent[0:HALF] = d0[0:HALF] + d1[HALF:L]
        dent[HALF:L] = d0[HALF:L] + d1[0:HALF]
        out[b] = (num / dent[None, :]).T.astype(np.float32)
    return out


def kernel(x, cache_k, cache_v, Wq, Wk, Wv):
    nc = build_program()
    in_maps = make_in_maps(x, cache_k, cache_v, Wq, Wk, Wv)
    results = bass2jax.run_bass_via_pjrt(nc, in_maps, n_cores=N_CORES)
    return combine(results)
